# revision 17
# baseline (speedup 1.0000x reference)
"""PointPillarScatter on 8 TRN2 cores via PE one-hot matmul, column-pair packed.

Scatter -> dense-matmul transform with two output columns packed per fp32
PSUM slot.  Core k owns flat canvas cols [k*88000, (k+1)*88000), padded to
88064 = 86 groups x 1024 cols.  A group is 4 "tile256"s (2 partition-stacks
x 2 free-segments) of 256 cols = 128 column-PAIRS; all of a group's pillars
(max 124 observed, budget 128) share one 128-slot contraction dim:

  lhsT = F [128 slots, 128]  col 64k+f: feat f of slots in stack k;
                             even-parity slots hold fp16(v), odd-parity
                             slots hold rint(v*256)*32 (exact in fp16)
  P [128 slots, 512] = is_equal(iota, pcol),
                             pcol = 256*parity + 128*seg + colpair
  psum[64k+f, 128*seg + c] = A + M*32  (A = even col value, M = rint(B*256))

One DVE tensor_scalar(is_equal) builds P per group; two matmuls per group
(N=256, shared lhsT) read P's parity halves and accumulate into the same
[128, 256] PSUM quarter -- each PSUM bank holds 2 groups as ONE
accumulation group (single start/stop per bank).  ScalarE copies packed
fp32 PSUM->SBUF (half the elements of the unpacked layout); DMA moves
fp32 pairs (same 11.3 MB/core as fp16 unpacked).  Host decodes
M = rint(x/32), A = x - 32M, B = M/256 (max abs err ~0.002 vs the 0.096
abs gate) and upcasts.
"""

import numpy as np

import concourse.bass as bass
import concourse.tile as tile
from concourse import mybir
from concourse.bass_utils import run_bass_kernel_spmd

NUM_FEATURES = 64
MAX_CAV = 5
NX, NY = 704, 200
NUM_PIXELS = NY * NX            # 140800
TOTAL = MAX_CAV * NUM_PIXELS    # 704000
N_CORES = 8
CORE_COLS = TOTAL // N_CORES    # 88000 flat columns per core
GROUPS = 86                     # groups of 1024 cols; 86*1024 = 88064 >= 88000
GCOLS = 1024
SLOTS = 128                     # slot budget per group (seed-0 max is 124)
PFREE = 512                     # P free dim: 2 parities x 2 segments x 128
QFREE = 256                     # psum free dim per group: 2 segments x 128
PAD_COLS = GROUPS * GCOLS       # 88064
OUT_W = GROUPS * QFREE          # 22016 packed fp32 per partition row
CHUNKS = [16, 16, 16, 16, 16, 4, 2]   # groups per stage tile / out-DMA
QUAD = 4                        # groups per PSUM tile (2 banks)

_PROG = None


def _split_excess_waits(nc, max_waits=1):
    """Walrus enforces tight per-instruction sync-wait encoding limits. Spill
    surplus waits onto single-wait EventSemaphore nops inserted just before
    the offending instruction on the same engine queue (same semantics:
    engine blocks at the nop, then proceeds)."""
    for blk in nc.main_func.blocks:
        i = 0
        while i < len(blk.instructions):
            inst = blk.instructions[i]
            si = inst.sync_info
            if si is None or len(si.on_wait) <= max_waits:
                i += 1
                continue
            waits = list(si.on_wait)
            keep, spill = waits[-max_waits:], waits[:-max_waits]
            for w in spill:
                nop = mybir.InstEventSemaphore(
                    name=f"I-{nc.next_id()}", ins=[], outs=[]
                )
                nop.engine = inst.engine
                nop.sync_info = mybir.SyncInfo(on_wait=[w], on_update=[])
                nc.register_instruction(nop)
                blk.instructions.insert(i, nop)
                i += 1
            si.on_wait = keep
            inst.sync_info = si
            i += 1


def _build_prog():
    f16 = mybir.dt.float16
    f32 = mybir.dt.float32
    nc = bass.Bass()
    feats = nc.dram_tensor("feats", [SLOTS, GROUPS * 128], f16, kind="ExternalInput")
    pcol = nc.dram_tensor("pcol", [SLOTS, GROUPS], f32, kind="ExternalInput")
    iota = nc.dram_tensor("iota", [SLOTS, PFREE], f16, kind="ExternalInput")
    out = nc.dram_tensor("out", [128, OUT_W], f32, kind="ExternalOutput")

    with tile.TileContext(nc) as tc:
        with (
            tc.tile_pool(name="const", bufs=1) as constp,
            tc.tile_pool(name="pmat", bufs=4) as pmatp,
            tc.tile_pool(name="psum", bufs=4, space="PSUM") as psump,
            tc.tile_pool(name="stage", bufs=3) as stagep,
        ):
            pcol_sb = constp.tile([SLOTS, GROUPS], f32)
            nc.sync.dma_start(pcol_sb[:], pcol[:])
            iota_sb = constp.tile([SLOTS, PFREE], f16)
            nc.sync.dma_start(iota_sb[:], iota[:])
            feats_sb = constp.tile([SLOTS, GROUPS * 128], f16)
            lo = 0
            for fg in (4, 21, 21, 20, 20):     # first chunk small: compute
                hi = lo + fg * 128             # on group 0 starts early
                nc.sync.dma_start(feats_sb[:, lo:hi], feats[:, lo:hi])
                lo = hi

            g0 = 0
            for ng in CHUNKS:
                st = stagep.tile([128, ng * QFREE], f32)
                for q in range(0, ng, QUAD):
                    ngt = min(QUAD, ng - q)
                    ps = psump.tile([128, ngt * QFREE], f32, space="PSUM")
                    for j in range(ngt):
                        g = g0 + q + j
                        P = pmatp.tile([SLOTS, PFREE], f16)
                        nc.vector.tensor_scalar(
                            out=P[:],
                            in0=iota_sb[:],
                            scalar1=pcol_sb[:, g:g + 1],
                            scalar2=None,
                            op0=mybir.AluOpType.is_equal,
                        )
                        lhsT = feats_sb[:, g * 128:(g + 1) * 128]
                        dst = ps[:, j * QFREE:(j + 1) * QFREE]
                        nc.tensor.matmul(
                            out=dst, lhsT=lhsT, rhs=P[:, 0:QFREE],
                            start=(j % 2 == 0), stop=False,
                        )
                        nc.tensor.matmul(
                            out=dst, lhsT=lhsT, rhs=P[:, QFREE:PFREE],
                            start=False, stop=(j % 2 == 1 or j == ngt - 1),
                        )
                    nc.scalar.activation(
                        st[:, q * QFREE:(q + ngt) * QFREE],
                        ps[:],
                        mybir.ActivationFunctionType.Copy,
                    )
                nc.sync.dma_start(
                    out[:, g0 * QFREE:(g0 + ng) * QFREE], st[:]
                )
                g0 += ng
    _split_excess_waits(nc)
    return nc


def _host_prep(voxel_coords, pillar_features):
    vc = voxel_coords.astype(np.int64)
    flat = vc[:, 0] * NUM_PIXELS + vc[:, 2] * NX + vc[:, 3]
    f32v = pillar_features.astype(np.float32)
    # parity-0 value: fp16(v); parity-1 value: rint(v*256)*32 (fp16-exact)
    feats_a = f32v.astype(np.float16)
    feats_b = (np.rint(f32v * 256.0) * 32.0).astype(np.float16)
    core = flat // CORE_COLS
    rem = flat - core * CORE_COLS
    g = rem // GCOLS
    w = rem - g * GCOLS
    t256 = w // 256                  # tile256 index 0..3
    k = t256 // 2                    # partition stack
    seg = t256 - 2 * k               # free segment
    w2 = w - t256 * 256
    cpair = w2 // 2                  # column pair 0..127
    parity = w2 - 2 * cpair
    j = 256 * parity + 128 * seg + cpair   # P free position [0, 512)
    lcol = 64 * k                    # lhsT column base (stack offset)

    # slot = rank of pillar within its (core, group)
    order = np.argsort(flat, kind="stable")
    gid_sorted = (core * GROUPS + g)[order]
    rank_sorted = np.arange(len(flat)) - np.searchsorted(
        gid_sorted, gid_sorted, side="left"
    )
    slot = np.empty(len(flat), np.int64)
    slot[order] = rank_sorted
    assert slot.max() < SLOTS, f"group overflow: {slot.max() + 1} slots"

    ar64 = np.arange(NUM_FEATURES)
    iota_arr = np.broadcast_to(
        np.arange(PFREE, dtype=np.float16), (SLOTS, PFREE)
    ).copy()
    feats_sel = np.where(parity[:, None] == 1, feats_b, feats_a)
    in_maps = []
    for cidx in range(N_CORES):
        m = core == cidx
        fa = np.zeros((SLOTS, GROUPS, 128), np.float16)
        pc = np.full((SLOTS, GROUPS), -1.0, np.float32)
        pc[slot[m], g[m]] = j[m].astype(np.float32)
        fa[slot[m][:, None], g[m][:, None], lcol[m][:, None] + ar64[None, :]] = (
            feats_sel[m]
        )
        in_maps.append({
            "feats": fa.reshape(SLOTS, GROUPS * 128),
            "pcol": pc,
            "iota": iota_arr,
        })
    return in_maps


def _unshard(core_outs):
    full = np.empty((TOTAL, NUM_FEATURES), np.float32)
    for cidx, o in enumerate(core_outs):       # o: [128, OUT_W] packed fp32
        M = np.rint(o * (1.0 / 32.0))
        A = o - M * 32.0                       # even-parity column values
        B = M * (1.0 / 256.0)                  # odd-parity column values
        # [p=2k x 64f, w=86g x 2seg x 128c] -> [g, k, seg, c, parity, f]
        r = np.stack([A, B], axis=-1).reshape(2, NUM_FEATURES, GROUPS, 2, 128, 2)
        r = r.transpose(2, 0, 3, 4, 5, 1).reshape(PAD_COLS, NUM_FEATURES)
        full[cidx * CORE_COLS:(cidx + 1) * CORE_COLS] = r[:CORE_COLS]
    return np.ascontiguousarray(
        full.reshape(MAX_CAV, NUM_PIXELS, NUM_FEATURES)
        .transpose(0, 2, 1)
        .reshape(MAX_CAV, NUM_FEATURES, NY, NX)
    )


def kernel(voxel_coords, pillar_features):
    global _PROG
    if _PROG is None:
        _PROG = _build_prog()
    in_maps = _host_prep(voxel_coords, pillar_features)
    res = run_bass_kernel_spmd(_PROG, in_maps, list(range(N_CORES)))
    return _unshard([r["out"] for r in res.results])


# revision 18
# speedup vs baseline: 1.0606x; 1.0606x over previous
"""PointPillarScatter on 8 TRN2 cores via PE one-hot matmul, column-pair packed.

Scatter -> dense-matmul transform with two output columns packed per fp32
PSUM slot.  Core k owns flat canvas cols [k*88000, (k+1)*88000), padded to
88064 = 86 groups x 1024 cols.  A group is 4 "tile256"s (2 partition-stacks
x 2 free-segments) of 256 cols = 128 column-PAIRS; all of a group's pillars
(max 124 observed, budget 128) share one 128-slot contraction dim:

  lhsT = F [128 slots, 128]  col 64k+f: feat f of slots in stack k;
                             even-parity slots hold fp16(v), odd-parity
                             slots hold rint(v*256)*32 (exact in fp16)
  P [128 slots, 512] = is_equal(iota, pcol),
                             pcol = 256*parity + 128*seg + colpair
  psum[64k+f, 128*seg + c] = A + M*32  (A = even col value, M = rint(B*256))

One DVE tensor_scalar(is_equal) builds P per group; two matmuls per group
(N=256, shared lhsT) read P's parity halves and accumulate into the same
[128, 256] PSUM quarter -- each PSUM bank holds 2 groups as ONE
accumulation group (single start/stop per bank).  ScalarE copies packed
fp32 PSUM->SBUF (half the elements of the unpacked layout); DMA moves
fp32 pairs (same 11.3 MB/core as fp16 unpacked).  Host decodes
M = rint(x/32), A = x - 32M, B = M/256 (max abs err ~0.002 vs the 0.096
abs gate) and upcasts.
"""

import numpy as np

import concourse.bass as bass
import concourse.tile as tile
from concourse import mybir
from concourse.bass_utils import run_bass_kernel_spmd

NUM_FEATURES = 64
MAX_CAV = 5
NX, NY = 704, 200
NUM_PIXELS = NY * NX            # 140800
TOTAL = MAX_CAV * NUM_PIXELS    # 704000
N_CORES = 8
CORE_COLS = TOTAL // N_CORES    # 88000 flat columns per core
GROUPS = 86                     # groups of 1024 cols; 86*1024 = 88064 >= 88000
GCOLS = 1024
SLOTS = 128                     # slot budget per group (seed-0 max is 124)
PFREE = 512                     # P free dim: 2 parities x 2 segments x 128
QFREE = 256                     # psum free dim per group: 2 segments x 128
PAD_COLS = GROUPS * GCOLS       # 88064
OUT_W = GROUPS * QFREE          # 22016 packed fp32 per partition row
CHUNKS = [16, 16, 16, 16, 16, 4, 2]   # groups per stage tile / out-DMA
QUAD = 4                        # groups per PSUM tile (2 banks)

_PROG = None


def _split_excess_waits(nc, max_waits=1):
    """Walrus enforces tight per-instruction sync-wait encoding limits. Spill
    surplus waits onto single-wait EventSemaphore nops inserted just before
    the offending instruction on the same engine queue (same semantics:
    engine blocks at the nop, then proceeds)."""
    for blk in nc.main_func.blocks:
        i = 0
        while i < len(blk.instructions):
            inst = blk.instructions[i]
            si = inst.sync_info
            if si is None or len(si.on_wait) <= max_waits:
                i += 1
                continue
            waits = list(si.on_wait)
            keep, spill = waits[-max_waits:], waits[:-max_waits]
            for w in spill:
                nop = mybir.InstEventSemaphore(
                    name=f"I-{nc.next_id()}", ins=[], outs=[]
                )
                nop.engine = inst.engine
                nop.sync_info = mybir.SyncInfo(on_wait=[w], on_update=[])
                nc.register_instruction(nop)
                blk.instructions.insert(i, nop)
                i += 1
            si.on_wait = keep
            inst.sync_info = si
            i += 1


def _build_prog():
    f16 = mybir.dt.float16
    f32 = mybir.dt.float32
    nc = bass.Bass()
    feats = nc.dram_tensor("feats", [SLOTS, GROUPS * 128], f16, kind="ExternalInput")
    pcol = nc.dram_tensor("pcol", [SLOTS, GROUPS], f32, kind="ExternalInput")
    iota = nc.dram_tensor("iota", [SLOTS, PFREE], f16, kind="ExternalInput")
    out = nc.dram_tensor("out", [128, OUT_W], f32, kind="ExternalOutput")

    with tile.TileContext(nc) as tc:
        with (
            tc.tile_pool(name="const", bufs=1) as constp,
            tc.tile_pool(name="pmat", bufs=4) as pmatp,
            tc.tile_pool(name="psum", bufs=4, space="PSUM") as psump,
            tc.tile_pool(name="stage", bufs=3) as stagep,
        ):
            pcol_sb = constp.tile([SLOTS, GROUPS], f32)
            nc.sync.dma_start(pcol_sb[:], pcol[:])
            iota_sb = constp.tile([SLOTS, PFREE], f16)
            nc.sync.dma_start(iota_sb[:], iota[:])
            feats_sb = constp.tile([SLOTS, GROUPS * 128], f16)
            FCH = 22 * 128
            for lo in range(0, GROUPS * 128, FCH):
                hi = min(lo + FCH, GROUPS * 128)
                nc.sync.dma_start(feats_sb[:, lo:hi], feats[:, lo:hi])

            g0 = 0
            for ng in CHUNKS:
                st = stagep.tile([128, ng * QFREE], f32)
                for q in range(0, ng, QUAD):
                    ngt = min(QUAD, ng - q)
                    ps = psump.tile([128, ngt * QFREE], f32, space="PSUM")
                    for j in range(ngt):
                        g = g0 + q + j
                        P = pmatp.tile([SLOTS, PFREE], f16)
                        nc.vector.tensor_scalar(
                            out=P[:],
                            in0=iota_sb[:],
                            scalar1=pcol_sb[:, g:g + 1],
                            scalar2=None,
                            op0=mybir.AluOpType.is_equal,
                        )
                        lhsT = feats_sb[:, g * 128:(g + 1) * 128]
                        dst = ps[:, j * QFREE:(j + 1) * QFREE]
                        nc.tensor.matmul(
                            out=dst, lhsT=lhsT, rhs=P[:, 0:QFREE],
                            start=(j % 2 == 0), stop=False,
                        )
                        nc.tensor.matmul(
                            out=dst, lhsT=lhsT, rhs=P[:, QFREE:PFREE],
                            start=False, stop=(j % 2 == 1 or j == ngt - 1),
                        )
                    nc.scalar.activation(
                        st[:, q * QFREE:(q + ngt) * QFREE],
                        ps[:],
                        mybir.ActivationFunctionType.Copy,
                    )
                nc.sync.dma_start(
                    out[:, g0 * QFREE:(g0 + ng) * QFREE], st[:]
                )
                g0 += ng
    _split_excess_waits(nc)
    return nc


def _host_prep(voxel_coords, pillar_features):
    vc = voxel_coords.astype(np.int64)
    flat = vc[:, 0] * NUM_PIXELS + vc[:, 2] * NX + vc[:, 3]
    f32v = pillar_features.astype(np.float32)
    # parity-0 value: fp16(v); parity-1 value: rint(v*256)*32 (fp16-exact)
    feats_a = f32v.astype(np.float16)
    feats_b = (np.rint(f32v * 256.0) * 32.0).astype(np.float16)
    core = flat // CORE_COLS
    rem = flat - core * CORE_COLS
    g = rem // GCOLS
    w = rem - g * GCOLS
    t256 = w // 256                  # tile256 index 0..3
    k = t256 // 2                    # partition stack
    seg = t256 - 2 * k               # free segment
    w2 = w - t256 * 256
    cpair = w2 // 2                  # column pair 0..127
    parity = w2 - 2 * cpair
    j = 256 * parity + 128 * seg + cpair   # P free position [0, 512)
    lcol = 64 * k                    # lhsT column base (stack offset)

    # slot = rank of pillar within its (core, group)
    order = np.argsort(flat, kind="stable")
    gid_sorted = (core * GROUPS + g)[order]
    rank_sorted = np.arange(len(flat)) - np.searchsorted(
        gid_sorted, gid_sorted, side="left"
    )
    slot = np.empty(len(flat), np.int64)
    slot[order] = rank_sorted
    assert slot.max() < SLOTS, f"group overflow: {slot.max() + 1} slots"

    ar64 = np.arange(NUM_FEATURES)
    iota_arr = np.broadcast_to(
        np.arange(PFREE, dtype=np.float16), (SLOTS, PFREE)
    ).copy()
    feats_sel = np.where(parity[:, None] == 1, feats_b, feats_a)
    in_maps = []
    for cidx in range(N_CORES):
        m = core == cidx
        fa = np.zeros((SLOTS, GROUPS, 128), np.float16)
        pc = np.full((SLOTS, GROUPS), -1.0, np.float32)
        pc[slot[m], g[m]] = j[m].astype(np.float32)
        fa[slot[m][:, None], g[m][:, None], lcol[m][:, None] + ar64[None, :]] = (
            feats_sel[m]
        )
        in_maps.append({
            "feats": fa.reshape(SLOTS, GROUPS * 128),
            "pcol": pc,
            "iota": iota_arr,
        })
    return in_maps


def _unshard(core_outs):
    full = np.empty((TOTAL, NUM_FEATURES), np.float32)
    for cidx, o in enumerate(core_outs):       # o: [128, OUT_W] packed fp32
        M = np.rint(o * (1.0 / 32.0))
        A = o - M * 32.0                       # even-parity column values
        B = M * (1.0 / 256.0)                  # odd-parity column values
        # [p=2k x 64f, w=86g x 2seg x 128c] -> [g, k, seg, c, parity, f]
        r = np.stack([A, B], axis=-1).reshape(2, NUM_FEATURES, GROUPS, 2, 128, 2)
        r = r.transpose(2, 0, 3, 4, 5, 1).reshape(PAD_COLS, NUM_FEATURES)
        full[cidx * CORE_COLS:(cidx + 1) * CORE_COLS] = r[:CORE_COLS]
    return np.ascontiguousarray(
        full.reshape(MAX_CAV, NUM_PIXELS, NUM_FEATURES)
        .transpose(0, 2, 1)
        .reshape(MAX_CAV, NUM_FEATURES, NY, NX)
    )


def kernel(voxel_coords, pillar_features):
    global _PROG
    if _PROG is None:
        _PROG = _build_prog()
    in_maps = _host_prep(voxel_coords, pillar_features)
    res = run_bass_kernel_spmd(_PROG, in_maps, list(range(N_CORES)))
    return _unshard([r["out"] for r in res.results])
